# revision 15
# baseline (speedup 1.0000x reference)
"""Multi-head attention (B=2, S=2048, H=2048, 16 heads, RoPE, causal) on 8 TRN2 cores.

Sharding: 2 batches x 4 head-groups (4 heads each). Core c handles batch c//4,
heads [4*(c%4) .. 4*(c%4)+3]. Host sums the 4 partial outputs per batch.

Design (vs the fp32 streaming baseline, ~475us/rep -> ~370us/rep measured):
  - bf16 everywhere off-PSUM (x, weights, q/k/v, probs, cos/sin, outputs);
    all matmul accumulation stays fp32 in PSUM. rel err ~4e-3 (gate 2e-2).
  - q/k/v/o weights resident in SBUF, loaded once in the prologue across two
    HWDGE rings; per-invocation DMA drops from ~100MB to ~27MB.
  - software pipeline over token-quarters: attention(t) is emitted interleaved
    with the q/k/v sweeps of quarter t+1, so the PE fills softmax-chain stalls
    with projection matmuls. PSUM: 2 sweep accumulators + 2 score pairs
    (2 banks each) + 1 pv + 1 den = 8 banks.
  - causal-tight diagonals: diagonal k-tiles only compute the q-range they can
    see; only the first 128 columns get the triangular mask (bf16 DVE mul).
  - scores go to 2-bank PSUM pairs, one exp instruction per pair (halves the
    ACT fixed overhead); probs come out bf16.
  - denominator = ones-column matmuls accumulated in PSUM (partition
    reduction is PE-only); full-width score pairs are first pair-summed on
    the DVE (bf16) so one ones-matmul covers two k-tiles (deferred two
    pairs so the add never stalls the PE); reciprocal via
    the fast Newton-Raphson custom op; broadcast back with a rank-1 matmul.
  - last quarter (no sweeps left to overlap) runs head pairs on alternating
    PSUM banks with their k-loops interleaved, and splits each score pair's
    exp into per-bank calls to shorten the exp->pv handoff.
"""

import numpy as np

import concourse.bass as bass
import concourse.mybir as mybir
import concourse.tile as tile
from concourse import bacc
from concourse.bass import ds, ts
from concourse.bass_utils import run_bass_kernel_spmd

F32 = mybir.dt.float32
F32R = mybir.dt.float32r
BF16 = mybir.dt.bfloat16

B, S, H, NH, HD = 2, 2048, 2048, 16, 128
NG = 4                 # head groups (cores per batch)
HPG = NH // NG         # heads per group = 4
GD = HPG * HD          # group width = 512
NQ = 4                 # t-quarters
QT = S // NQ           # 512 tokens per quarter
HC = H // 128          # 16 contraction chunks
SCALE = float(HD) ** -0.5
EXP = mybir.ActivationFunctionType.Exp


def build(reps: int = 1):
    nc = bacc.Bacc("TRN2", target_bir_lowering=False, debug=False, num_devices=8)
    xq = nc.dram_tensor("xq", [NQ, H, QT], BF16, kind="ExternalInput").ap()
    wqt = nc.dram_tensor("wqt", [H, GD], BF16, kind="ExternalInput").ap()
    wkt = nc.dram_tensor("wkt", [H, GD], BF16, kind="ExternalInput").ap()
    wvt = nc.dram_tensor("wvt", [H, GD], BF16, kind="ExternalInput").ap()
    wot = nc.dram_tensor("wot", [NQ, GD, QT], BF16, kind="ExternalInput").ap()
    cosd = nc.dram_tensor("cosd", [128, S], BF16, kind="ExternalInput").ap()
    sind = nc.dram_tensor("sind", [128, S], BF16, kind="ExternalInput").ap()
    maskd = nc.dram_tensor("maskd", [128, 128], BF16, kind="ExternalInput").ap()
    onesd = nc.dram_tensor("onesd", [128, 1], F32R, kind="ExternalInput").ap()
    onesbd = nc.dram_tensor("onesbd", [128, 1], BF16, kind="ExternalInput").ap()
    outq = nc.dram_tensor("outq", [NQ, S, QT], BF16, kind="ExternalOutput").ap()

    with tile.TileContext(nc) as tc:
        with (
            nc.allow_low_precision(reason="bf16 pipeline with fp32 accumulation"),
            tc.tile_pool(name="res", bufs=1) as res,          # persistents
            tc.tile_pool(name="xp", bufs=7) as xp,            # x quarter-chunks
            tc.tile_pool(name="qtp", bufs=1) as qtp,          # qT per head
            tc.tile_pool(name="ptp", bufs=4) as ptp,          # prob pairs
            tc.tile_pool(name="pmp", bufs=2) as pmp,          # masked prob pairs
            tc.tile_pool(name="rp", bufs=3) as rp,            # rope temps + misc
            tc.tile_pool(name="atp", bufs=2) as atp,          # attnT per head
            tc.tile_pool(name="osb", bufs=2) as osb,          # out staging
            tc.tile_pool(name="ps_acc", bufs=2, space="PSUM") as ps_acc,
            tc.tile_pool(name="ps_sc", bufs=2, space="PSUM") as ps_sc,
            tc.tile_pool(name="ps_pv", bufs=1, space="PSUM") as ps_pv,
            tc.tile_pool(name="ps_den", bufs=1, space="PSUM") as ps_den,
        ):
            # ---- persistent tiles ----
            NWP = 4                # weight pieces (fine-grained prologue DMA)
            WPC = HC // NWP        # contraction chunks per piece = 4
            wq_r = [
                res.tile([128, WPC, GD], BF16, tag=f"wq{i}", name=f"wq{i}")
                for i in range(NWP)
            ]
            wk_r = [
                res.tile([128, WPC, GD], BF16, tag=f"wk{i}", name=f"wk{i}")
                for i in range(NWP)
            ]
            wv_r = [
                res.tile([128, WPC, GD], BF16, tag=f"wv{i}", name=f"wv{i}")
                for i in range(NWP)
            ]
            wo_r = res.tile([128, NQ, HPG, QT], BF16, tag="wo", name="wo")
            cos_t = res.tile([128, S], BF16, tag="cos", name="cos")
            sin_t = res.tile([128, S], BF16, tag="sin", name="sin")
            mask_t = res.tile([128, 128], BF16, tag="mask", name="mask")
            ones_c = res.tile([128, 1], BF16, tag="onesc", name="onesc")
            ones_r = res.tile([1, 128], F32R, tag="onesr", name="onesr")
            kT = [
                [res.tile([128, QT], BF16, tag=f"kT{h}_{q}", name=f"kT{h}_{q}") for q in range(NQ)]
                for h in range(HPG)
            ]
            vres = [res.tile([128, GD], BF16, tag=f"v{i}", name=f"v{i}") for i in range(S // 128)]

            def load_weight(dst, dram, piece):
                # one quarter of a [H, GD] weight as [128, HC/4, GD] into its
                # own tile (separate tiles keep dependency tracking
                # fine-grained so the first sweep starts after ~0.5MB).
                # scalar-engine HWDGE ring: keeps the x loads (sync ring) from
                # queueing behind 9MB of resident-weight traffic.
                nc.scalar.dma_start(
                    dst[piece][:],
                    dram[ds(piece * WPC * 128, WPC * 128), :].rearrange(
                        "(n p) o -> p n o", p=128
                    ),
                )

            def load_x(tcq, c, eng=None):
                # 4 H-chunks of this quarter's x^T: [128, 4, QT]
                xt = xp.tile([128, 4, QT], BF16, tag="xq", name="xq")
                (eng or nc.sync).dma_start(
                    xt[:],
                    xq[tcq].rearrange("(n p) t -> p n t", p=128)[
                        :, ds(c * 4, 4), :
                    ],
                )
                return xt

            # ---- prologue: weights + constants (once, outside reps loop).
            # Order matters: each piece lands just before its first use
            # (q-sweep -> rope -> k-sweep -> v-sweep -> outproj).
            for p in range(NWP):
                load_weight(wq_r, wqt, p)

            def load_consts_weights():
                nc.gpsimd.dma_start(cos_t[:], cosd)
                nc.gpsimd.dma_start(sin_t[:], sind)
                for p in range(NWP):
                    load_weight(wk_r, wkt, p)
                for p in range(NWP):
                    load_weight(wv_r, wvt, p)
                nc.gpsimd.dma_start(mask_t[:], maskd)
                nc.gpsimd.dma_start(ones_c[:], onesbd)
                nc.gpsimd.dma_start(ones_r[:], onesd.rearrange("p o -> o p"))
                nc.scalar.dma_start(
                    wo_r[:],
                    wot.rearrange("oc (h p) t -> p oc h t", p=128),
                )

            def rope(acc_ps, tsl, out_ap):
                # out = acc*cos + rotate_half(acc)*sin   (sin table pre-negated
                # in rows 64:128). DVE copy frees the PSUM bank; GPSIMD does the
                # cross-partition halves.
                asb = rp.tile([128, QT], F32, tag="asb", name="asb")
                nc.vector.tensor_copy(asb[:], acc_ps[:])
                t1 = rp.tile([128, QT], F32, tag="r1", name="r1")
                nc.vector.tensor_mul(t1[:], asb[:], cos_t[:, tsl])
                t2 = rp.tile([128, QT], F32, tag="r2", name="r2")
                nc.gpsimd.tensor_mul(t2[0:64, :], asb[64:128, :], sin_t[64:128, tsl])
                nc.gpsimd.tensor_mul(t2[64:128, :], asb[0:64, :], sin_t[0:64, tsl])
                nc.vector.tensor_add(out_ap, t1[:], t2[:])

            def sweep_groups(tcq, xs, qT_out):
                """Closures, each emitting one 32-matmul sweep group + evac.
                Running all of them computes q/k/v projections for quarter tcq;
                qT_out[h] is filled as the q-groups run."""

                def xslice(hc):
                    return xs[hc // 4][:, hc % 4, :]

                tsl = ts(tcq, QT)
                groups = []
                for w_r, is_q in ((wq_r, True), (wk_r, False)):
                    for hp in (0, 2):
                        def qk_group(w_r=w_r, is_q=is_q, hp=hp):
                            accs = [
                                ps_acc.tile([128, QT], F32, tag="acc", name="acc")
                                for _ in range(2)
                            ]
                            for hc in range(HC):
                                for i in range(2):
                                    h = hp + i
                                    nc.tensor.matmul(
                                        accs[i][:],
                                        w_r[hc // WPC][:, hc % WPC, ds(h * 128, 128)],
                                        xslice(hc),
                                        start=(hc == 0),
                                        stop=(hc == HC - 1),
                                    )
                            for i in range(2):
                                h = hp + i
                                if is_q:
                                    qt_t = qtp.tile(
                                        [128, QT], BF16, tag=f"q{h}", name=f"q{h}"
                                    )
                                    rope(accs[i], tsl, qt_t[:])
                                    qT_out[h] = qt_t
                                else:
                                    rope(accs[i], tsl, kT[h][tcq][:])

                        groups.append(qk_group)
                for tp in (0, 2):
                    def v_group(tp=tp):
                        accs = [
                            ps_acc.tile([128, GD], F32, tag="acc", name="acc")
                            for _ in range(2)
                        ]
                        for hc in range(HC):
                            for i in range(2):
                                nc.tensor.matmul(
                                    accs[i][:],
                                    xslice(hc)[:, ts(tp + i, 128)],
                                    wv_r[hc // WPC][:, hc % WPC, :],
                                    start=(hc == 0),
                                    stop=(hc == HC - 1),
                                )
                        for i in range(2):
                            nc.vector.tensor_copy(
                                vres[tcq * 4 + tp + i][:], accs[i][:]
                            )

                    groups.append(v_group)
                return groups

            def attn_head(tcq, h, qT, attnT, pv_pool, den_pool):
                for _ in attn_head_gen(tcq, h, qT, attnT, pv_pool, den_pool):
                    pass

            def attn_head_gen(tcq, h, qT, attnT, pv_pool, den_pool):
                # Causal-tight: diagonal k-tile ki=4*tcq+j only attends
                # q-cols [128j, 512) of the quarter; within that only the
                # first 128 cols need the triangular mask.
                nki = 4 * (tcq + 1)
                pv_ps = pv_pool.tile([128, QT], F32, tag=pv_pool._attag, name="pv")
                den_ps = den_pool.tile([1, QT], F32, tag=den_pool._attag, name="den")
                first = [True]
                first_den = [True]
                pending_den = []

                def pv_den(ki, src_ap, col0, ncols, stop):
                    nc.tensor.matmul(
                        pv_ps[:, ds(col0, ncols)],
                        vres[ki][:, ts(h, 128)],
                        src_ap,
                        start=first[0],
                        stop=stop,
                    )
                    first[0] = False
                    nc.tensor.matmul(
                        den_ps[:, ds(col0, ncols)],
                        ones_c[:],
                        src_ap,
                        start=first_den[0],
                        stop=stop,
                    )
                    first_den[0] = False

                pending_tsum = []

                def emit_den_for(tsum):
                    def emit():
                        nc.tensor.matmul(
                            den_ps[:],
                            ones_c[:],
                            tsum[:],
                            start=first_den[0],
                            stop=False,
                        )
                        first_den[0] = False

                    return emit

                kps = list(range(nki // 2))
                for kp in kps:
                    yield
                    if kp < 2 * tcq:  # full pair
                        sc2 = ps_sc.tile([128, 2 * QT], F32, tag="sc", name="sc")
                        pt2 = ptp.tile([128, 2 * QT], BF16, tag="pt", name="pt")
                        split_exp = tcq == NQ - 1
                        for half in range(2):
                            ki = kp * 2 + half
                            nc.tensor.matmul(
                                sc2[:, ts(half, QT)],
                                kT[h][ki // 4][:, ts(ki % 4, 128)],
                                qT[h][:],
                                start=True,
                                stop=True,
                            )
                            if split_exp:
                                nc.scalar.activation(
                                    pt2[:, ts(half, QT)],
                                    sc2[:, ts(half, QT)],
                                    EXP,
                                    scale=SCALE,
                                )
                        if not split_exp:
                            nc.scalar.activation(pt2[:], sc2[:], EXP, scale=SCALE)
                        for half in range(2):
                            ki = kp * 2 + half
                            nc.tensor.matmul(
                                pv_ps[:],
                                vres[ki][:, ts(h, 128)],
                                pt2[:, ts(half, QT)],
                                start=first[0],
                                stop=False,
                            )
                            first[0] = False
                        tsum = pmp.tile([128, QT], BF16, tag="tsum", name="tsum", bufs=4)
                        nc.vector.tensor_add(
                            tsum[:], pt2[:, ts(0, QT)], pt2[:, ts(1, QT)]
                        )
                        # merge two pair-sums (4 k-tiles) per den matmul;
                        # defer so the DVE adds never stall the PE
                        pending_tsum.append(tsum)
                        if len(pending_tsum) >= 3:
                            ta = pending_tsum.pop(0)
                            tb = pending_tsum.pop(0)
                            t4 = pmp.tile(
                                [128, QT], BF16, tag="tsum4", name="tsum4", bufs=2
                            )
                            nc.vector.tensor_add(t4[:], ta[:], tb[:])
                            pending_den.append(emit_den_for(t4))
                        if len(pending_den) > 1:
                            pending_den.pop(0)()
                    else:  # diagonal pair: j pairs (0,1) then (2,3)
                        if pending_tsum:
                            if len(pending_tsum) == 2:
                                ta = pending_tsum.pop(0)
                                tb = pending_tsum.pop(0)
                                t4 = pmp.tile(
                                    [128, QT], BF16, tag="tsum4", name="tsum4", bufs=2
                                )
                                nc.vector.tensor_add(t4[:], ta[:], tb[:])
                                pending_den.append(emit_den_for(t4))
                            else:
                                pending_den.append(emit_den_for(pending_tsum.pop(0)))
                        while pending_den:
                            pending_den.pop(0)()
                        ja = 2 * (kp - 2 * tcq)  # 0 or 2
                        wa, wb = QT - 128 * ja, QT - 128 * (ja + 1)
                        sc2 = ps_sc.tile([128, wa + wb], F32, tag="sc", name="sc")
                        nc.tensor.matmul(
                            sc2[:, 0:wa],
                            kT[h][tcq][:, ts(ja, 128)],
                            qT[h][:, ds(128 * ja, wa)],
                            start=True,
                            stop=True,
                        )
                        nc.tensor.matmul(
                            sc2[:, ds(wa, wb)],
                            kT[h][tcq][:, ts(ja + 1, 128)],
                            qT[h][:, ds(128 * (ja + 1), wb)],
                            start=True,
                            stop=True,
                        )
                        pt2 = ptp.tile([128, wa + wb], BF16, tag="pt", name="pt")
                        nc.scalar.activation(pt2[:], sc2[:], EXP, scale=SCALE)
                        # mask the first 128 cols of each diagonal piece
                        # in place, then one pv + one den matmul per piece
                        for j, off, w in ((ja, 0, wa), (ja + 1, wa, wb)):
                            nc.vector.tensor_mul(
                                pt2[:, ds(off, 128)],
                                pt2[:, ds(off, 128)],
                                mask_t[:, 0:128],
                            )
                        for j, off, w in ((ja, 0, wa), (ja + 1, wa, wb)):
                            ki = 4 * tcq + j
                            pv_den(ki, pt2[:, ds(off, w)], 128 * j, w, stop=(j == 3))
                recip = rp.tile([1, QT], F32, tag="rc", name="rc")
                nc.vector.reciprocal_approx_fast(recip[:], den_ps[:])
                recip_r = rp.tile([1, QT], F32R, tag="rcr", name="rcr")
                nc.vector.tensor_copy(recip_r[:], recip[:])
                bc = den_pool.tile([128, QT], F32, tag=den_pool._attag, name="bc")
                nc.tensor.matmul(
                    bc[:], ones_r[:], recip_r[:], start=True, stop=True
                )
                bc_sb = rp.tile([128, QT], F32, tag="bcsb", name="bcsb")
                nc.vector.tensor_copy(bc_sb[:], bc[:])
                at_t = atp.tile([128, QT], BF16, tag=f"at{h}", name=f"at{h}")
                nc.vector.tensor_mul(at_t[:], pv_ps[:], bc_sb[:])
                while len(attnT) <= h:
                    attnT.append(None)
                attnT[h] = at_t

            ps_pv._attag = "pv"
            ps_den._attag = "den"
            ps_acc._attag = "acc"

            # ---- software pipeline over quarters: attention(t) interleaves
            # with the projection sweeps of quarter t+1 on the PE stream ----
            quarters = [(r, t) for r in range(reps) for t in range(NQ)]
            xs_cur = [
                load_x(0, c, eng=(nc.gpsimd if c % 2 else nc.sync))
                for c in range(4)
            ]
            load_consts_weights()
            xs_pre1 = (
                [load_x(1, c) for c in range(4)] if len(quarters) > 1 else None
            )
            qT_cur = {}
            for g in sweep_groups(0, xs_cur, qT_cur):
                g()
            for qi, (rep, tcq) in enumerate(quarters):
                tsl = ts(tcq, QT)
                last = qi == len(quarters) - 1
                if not last:
                    ntc = quarters[qi + 1][1]
                    xs_next = (
                        xs_pre1 if qi == 0 else [load_x(ntc, c) for c in range(4)]
                    )
                    qT_next = {}
                    groups = sweep_groups(ntc, xs_next, qT_next)
                else:
                    groups = []

                attnT = []
                if not last:
                    for h in range(HPG):
                        attn_head(tcq, h, qT_cur, attnT, ps_pv, ps_den)
                        groups[h]()
                else:
                    # final quarter: no sweeps to overlap; run head pairs on
                    # alternating PSUM banks (borrowing the freed sweep banks)
                    # with their k-loops interleaved so neither head's PE work
                    # stalls on its own exp handoff
                    for hp in (0, 2):
                        gens = [
                            attn_head_gen(tcq, hp, qT_cur, attnT, ps_pv, ps_den),
                            attn_head_gen(
                                tcq, hp + 1, qT_cur, attnT, ps_acc, ps_acc
                            ),
                        ]
                        while gens:
                            for g in list(gens):
                                try:
                                    next(g)
                                except StopIteration:
                                    gens.remove(g)

                # ---- output projection, remaining sweep groups in between ----
                for oc in range(NQ):
                    ost = osb.tile([128, 4, QT], BF16, tag="ost", name="ost")
                    for tt in range(4):
                        ops = ps_sc.tile([128, QT], F32, tag="sc", name="sc")
                        for h in range(HPG):
                            nc.tensor.matmul(
                                ops[:],
                                attnT[h][:, ts(tt, 128)],
                                wo_r[:, oc, h, :],
                                start=(h == 0),
                                stop=(h == HPG - 1),
                            )
                        nc.scalar.copy(ost[:, tt, :], ops[:])
                    nc.sync.dma_start(
                        outq[oc, tsl, :].rearrange("(tt p) o -> p tt o", p=128),
                        ost[:],
                    )
                    if not last and oc < 2:
                        groups[HPG + oc]()

                if not last:
                    xs_cur, qT_cur = xs_next, qT_next
    nc.finalize()
    return nc


def _host_tables():
    import ml_dtypes

    inv = 1.0 / (10000.0 ** (np.arange(64, dtype=np.float64) / 64.0))
    ang = inv[:, None] * np.arange(S, dtype=np.float64)[None, :]  # [64, S]
    cosL = np.cos(ang)
    sinL = np.sin(ang)
    cos_t = np.vstack([cosL, cosL]).astype(ml_dtypes.bfloat16)
    sin_t = np.vstack([sinL, -sinL]).astype(ml_dtypes.bfloat16)
    kp = np.arange(128)[:, None]
    qf = np.arange(128)[None, :]
    mask = (qf >= kp).astype(ml_dtypes.bfloat16)
    ones = np.ones((128, 1), np.float32)
    return cos_t, sin_t, mask, ones


def _make_in_maps(hidden_states, wq, wk, wv, wo):
    import ml_dtypes

    bf16 = ml_dtypes.bfloat16
    x = np.asarray(hidden_states, dtype=np.float32)
    wq = np.asarray(wq, dtype=np.float32)
    wk = np.asarray(wk, dtype=np.float32)
    wv = np.asarray(wv, dtype=np.float32)
    wo = np.asarray(wo, dtype=np.float32)
    cos_t, sin_t, mask, ones = _host_tables()
    onesb = ones.astype(bf16)
    xqa_by_b = [
        np.ascontiguousarray(
            x[b].T.reshape(H, NQ, QT).transpose(1, 0, 2)
        ).astype(bf16)
        for b in range(B)
    ]
    w_by_g = []
    for g in range(NG):
        sl = slice(GD * g, GD * (g + 1))
        w_by_g.append(
            (
                np.ascontiguousarray(wq[sl, :].T).astype(bf16),
                np.ascontiguousarray(wk[sl, :].T).astype(bf16),
                np.ascontiguousarray(wv[sl, :].T).astype(bf16),
                # wot[oc, d, o] = wo[oc*QT + o, GD*g + d]
                np.ascontiguousarray(
                    wo[:, sl].reshape(NQ, QT, GD).transpose(0, 2, 1)
                ).astype(bf16),
            )
        )
    in_maps = []
    for c in range(8):
        b, g = divmod(c, NG)
        wqt, wkt, wvt, wot = w_by_g[g]
        in_maps.append(
            {
                "xq": xqa_by_b[b], "wqt": wqt, "wkt": wkt, "wvt": wvt,
                "wot": wot, "cosd": cos_t, "sind": sin_t, "maskd": mask,
                "onesd": ones, "onesbd": onesb,
            }
        )
    return in_maps


def _gather(results, bo):
    out = np.zeros((B, S, H), dtype=np.float32)
    for c in range(8):
        b = c // NG
        oq = results[c]["outq"].astype(np.float32)  # [NQ, S, QT]
        out[b] += np.concatenate(list(oq), axis=1)
    out += np.asarray(bo, dtype=np.float32)[None, None, :]
    return out


def kernel(hidden_states, wq, bq, wk, bk, wv, bv, wo, bo):
    in_maps = _make_in_maps(hidden_states, wq, wk, wv, wo)
    nc = build()
    res = run_bass_kernel_spmd(nc, in_maps, core_ids=list(range(8)))
    return _gather(res.results, bo)



# revision 26
# speedup vs baseline: 3.5932x; 3.5932x over previous
"""Multi-head attention (B=2, S=2048, H=2048, 16 heads, RoPE, causal) on 8 TRN2 cores.

Sharding: 2 batches x 4 head-groups (4 heads each). Core c handles batch c//4,
heads [4*(c%4) .. 4*(c%4)+3]. Host sums the 4 partial outputs per batch.

Design (vs the fp32 streaming baseline, ~475us/rep -> ~370us/rep measured):
  - bf16 everywhere off-PSUM (x, weights, q/k/v, probs, cos/sin, outputs);
    all matmul accumulation stays fp32 in PSUM. rel err ~4e-3 (gate 2e-2).
  - q/k/v/o weights resident in SBUF, loaded once in the prologue across two
    HWDGE rings; per-invocation DMA drops from ~100MB to ~27MB.
  - software pipeline over token-quarters: attention(t) is emitted interleaved
    with the q/k/v sweeps of quarter t+1, so the PE fills softmax-chain stalls
    with projection matmuls. PSUM: 2 sweep accumulators + 2 score pairs
    (2 banks each) + 1 pv + 1 den = 8 banks.
  - causal-tight diagonals: diagonal k-tiles only compute the q-range they can
    see; only the first 128 columns get the triangular mask (bf16 DVE mul).
  - scores go to 2-bank PSUM pairs, one exp instruction per pair (halves the
    ACT fixed overhead); probs come out bf16.
  - denominator = ones-column matmuls accumulated in PSUM (partition
    reduction is PE-only); full-width score pairs are first pair-summed on
    the DVE (bf16) so one ones-matmul covers two k-tiles (deferred two
    pairs so the add never stalls the PE); reciprocal via
    the fast Newton-Raphson custom op; broadcast back with a rank-1 matmul.
  - last quarter (no sweeps left to overlap) runs head pairs on alternating
    PSUM banks with their k-loops interleaved, and splits each score pair's
    exp into per-bank calls to shorten the exp->pv handoff.
"""

import numpy as np

import concourse.bass as bass
import concourse.mybir as mybir
import concourse.tile as tile
from concourse import bacc
from concourse.bass import ds, ts
from concourse.bass_utils import run_bass_kernel_spmd

F32 = mybir.dt.float32
F32R = mybir.dt.float32r
BF16 = mybir.dt.bfloat16

B, S, H, NH, HD = 2, 2048, 2048, 16, 128
NG = 4                 # head groups (cores per batch)
HPG = NH // NG         # heads per group = 4
GD = HPG * HD          # group width = 512
NQ = 4                 # t-quarters
QT = S // NQ           # 512 tokens per quarter
HC = H // 128          # 16 contraction chunks
SCALE = float(HD) ** -0.5
EXP = mybir.ActivationFunctionType.Exp


def build(reps: int = 1):
    nc = bacc.Bacc("TRN2", target_bir_lowering=False, debug=False, num_devices=8)
    xq = nc.dram_tensor("xq", [NQ, H, QT], BF16, kind="ExternalInput").ap()
    wqt = nc.dram_tensor("wqt", [H, GD], BF16, kind="ExternalInput").ap()
    wkt = nc.dram_tensor("wkt", [H, GD], BF16, kind="ExternalInput").ap()
    wvt = nc.dram_tensor("wvt", [H, GD], BF16, kind="ExternalInput").ap()
    wot = nc.dram_tensor("wot", [NQ, GD, QT], BF16, kind="ExternalInput").ap()
    cosd = nc.dram_tensor("cosd", [128, S], BF16, kind="ExternalInput").ap()
    sind = nc.dram_tensor("sind", [128, S], BF16, kind="ExternalInput").ap()
    maskd = nc.dram_tensor("maskd", [128, 128], BF16, kind="ExternalInput").ap()
    onesd = nc.dram_tensor("onesd", [128, 1], F32R, kind="ExternalInput").ap()
    onesbd = nc.dram_tensor("onesbd", [128, 1], BF16, kind="ExternalInput").ap()
    outq = nc.dram_tensor("outq", [NQ, S, QT], BF16, kind="ExternalOutput").ap()

    with tile.TileContext(nc) as tc:
        with (
            nc.allow_low_precision(reason="bf16 pipeline with fp32 accumulation"),
            tc.tile_pool(name="res", bufs=1) as res,          # persistents
            tc.tile_pool(name="xp", bufs=7) as xp,            # x quarter-chunks
            tc.tile_pool(name="qtp", bufs=1) as qtp,          # qT per head
            tc.tile_pool(name="ptp", bufs=4) as ptp,          # prob pairs
            tc.tile_pool(name="pmp", bufs=2) as pmp,          # masked prob pairs
            tc.tile_pool(name="rp", bufs=3) as rp,            # rope temps + misc
            tc.tile_pool(name="atp", bufs=2) as atp,          # attnT per head
            tc.tile_pool(name="osb", bufs=2) as osb,          # out staging
            tc.tile_pool(name="ps_acc", bufs=2, space="PSUM") as ps_acc,
            tc.tile_pool(name="ps_sc", bufs=2, space="PSUM") as ps_sc,
            tc.tile_pool(name="ps_pv", bufs=1, space="PSUM") as ps_pv,
            tc.tile_pool(name="ps_den", bufs=1, space="PSUM") as ps_den,
        ):
            # ---- persistent tiles ----
            NWP = 4                # weight pieces (fine-grained prologue DMA)
            WPC = HC // NWP        # contraction chunks per piece = 4
            wq_r = [
                res.tile([128, WPC, GD], BF16, tag=f"wq{i}", name=f"wq{i}")
                for i in range(NWP)
            ]
            wk_r = [
                res.tile([128, WPC, GD], BF16, tag=f"wk{i}", name=f"wk{i}")
                for i in range(NWP)
            ]
            wv_r = [
                res.tile([128, WPC, GD], BF16, tag=f"wv{i}", name=f"wv{i}")
                for i in range(NWP)
            ]
            wo_r = res.tile([128, NQ, HPG, QT], BF16, tag="wo", name="wo")
            cos_h = [
                res.tile([128, 2 * QT], BF16, tag=f"cos{i}", name=f"cos{i}")
                for i in range(2)
            ]
            sin_h = [
                res.tile([128, 2 * QT], BF16, tag=f"sin{i}", name=f"sin{i}")
                for i in range(2)
            ]
            mask_t = res.tile([128, 128], BF16, tag="mask", name="mask")
            ones_c = res.tile([128, 1], BF16, tag="onesc", name="onesc")
            ones_r = res.tile([1, 128], F32R, tag="onesr", name="onesr")
            kT = [
                [res.tile([128, QT], BF16, tag=f"kT{h}_{q}", name=f"kT{h}_{q}") for q in range(NQ)]
                for h in range(HPG)
            ]
            vres = [res.tile([128, GD], BF16, tag=f"v{i}", name=f"v{i}") for i in range(S // 128)]

            def load_weight(dst, dram, piece):
                # one quarter of a [H, GD] weight as [128, HC/4, GD] into its
                # own tile (separate tiles keep dependency tracking
                # fine-grained so the first sweep starts after ~0.5MB).
                # scalar-engine HWDGE ring: keeps the x loads (sync ring) from
                # queueing behind 9MB of resident-weight traffic.
                nc.scalar.dma_start(
                    dst[piece][:],
                    dram[ds(piece * WPC * 128, WPC * 128), :].rearrange(
                        "(n p) o -> p n o", p=128
                    ),
                )

            def load_x(tcq, c, eng=None):
                # 4 H-chunks of this quarter's x^T: [128, 4, QT]
                xt = xp.tile([128, 4, QT], BF16, tag="xq", name="xq")
                (eng or nc.sync).dma_start(
                    xt[:],
                    xq[tcq].rearrange("(n p) t -> p n t", p=128)[
                        :, ds(c * 4, 4), :
                    ],
                )
                return xt

            # ---- prologue: weights + constants (once, outside reps loop).
            # The sim (and roughly the HW) serializes DMA on shared HBM
            # bandwidth, so global emission order ~= arrival order. Lead with
            # exactly what the first q-sweep needs (wq piece p paced against
            # x chunk c), then tables, then the rest.
            def load_consts_weights():
                nc.gpsimd.dma_start(cos_h[0][:], cosd[:, 0 : 2 * QT])
                nc.gpsimd.dma_start(sin_h[0][:], sind[:, 0 : 2 * QT])
                for p in range(NWP):
                    load_weight(wk_r, wkt, p)
                for p in range(NWP):
                    load_weight(wv_r, wvt, p)
                nc.gpsimd.dma_start(cos_h[1][:], cosd[:, 2 * QT : S])
                nc.gpsimd.dma_start(sin_h[1][:], sind[:, 2 * QT : S])
                nc.gpsimd.dma_start(mask_t[:], maskd)
                nc.gpsimd.dma_start(ones_c[:], onesbd)
                nc.gpsimd.dma_start(ones_r[:], onesd.rearrange("p o -> o p"))
                nc.scalar.dma_start(
                    wo_r[:],
                    wot.rearrange("oc (h p) t -> p oc h t", p=128),
                )

            def rope(acc_ps, tcq, out_ap):
                # out = acc*cos + rotate_half(acc)*sin   (sin table pre-negated
                # in rows 64:128). DVE only does the PSUM evac + cos mul (bf16,
                # 2x rate); the cross-partition halves AND the final add run on
                # GPSIMD so no DVE instruction ever waits on the pool engine
                # (the attention's mask-muls/pair-sums queue behind rope ops in
                # the DVE FIFO, so a pool-waiting add convoys the PE).
                cos_t = cos_h[tcq // 2]
                sin_t = sin_h[tcq // 2]
                tsl = ts(tcq % 2, QT)
                asb = rp.tile([128, QT], BF16, tag="asb", name="asb")
                nc.vector.tensor_copy(asb[:], acc_ps[:])
                t1 = rp.tile([128, QT], BF16, tag="r1", name="r1")
                nc.vector.tensor_mul(t1[:], asb[:], cos_t[:, tsl])
                t2 = rp.tile([128, QT], BF16, tag="r2", name="r2")
                nc.gpsimd.tensor_mul(t2[0:64, :], asb[64:128, :], sin_t[64:128, tsl])
                nc.gpsimd.tensor_mul(t2[64:128, :], asb[0:64, :], sin_t[0:64, tsl])
                nc.gpsimd.tensor_add(out_ap, t1[:], t2[:])

            def sweep_groups(tcq, xs, qT_out):
                """Closures, each emitting one 32-matmul sweep group + evac.
                Running all of them computes q/k/v projections for quarter tcq;
                qT_out[h] is filled as the q-groups run."""

                def xslice(hc):
                    return xs[hc // 4][:, hc % 4, :]

                groups = []
                for w_r, is_q in ((wq_r, True), (wk_r, False)):
                    for hp in (0, 2):
                        def qk_group(w_r=w_r, is_q=is_q, hp=hp):
                            accs = [
                                ps_acc.tile([128, QT], F32, tag="acc", name="acc")
                                for _ in range(2)
                            ]
                            for hc in range(HC):
                                for i in range(2):
                                    h = hp + i
                                    nc.tensor.matmul(
                                        accs[i][:],
                                        w_r[hc // WPC][:, hc % WPC, ds(h * 128, 128)],
                                        xslice(hc),
                                        start=(hc == 0),
                                        stop=(hc == HC - 1),
                                    )
                            for i in range(2):
                                h = hp + i
                                if is_q:
                                    qt_t = qtp.tile(
                                        [128, QT], BF16, tag=f"q{h}", name=f"q{h}"
                                    )
                                    rope(accs[i], tcq, qt_t[:])
                                    qT_out[h] = qt_t
                                else:
                                    rope(accs[i], tcq, kT[h][tcq][:])

                        groups.append(qk_group)
                for tp in (0, 2):
                    def v_group(tp=tp):
                        accs = [
                            ps_acc.tile([128, GD], F32, tag="acc", name="acc")
                            for _ in range(2)
                        ]
                        for hc in range(HC):
                            for i in range(2):
                                nc.tensor.matmul(
                                    accs[i][:],
                                    xslice(hc)[:, ts(tp + i, 128)],
                                    wv_r[hc // WPC][:, hc % WPC, :],
                                    start=(hc == 0),
                                    stop=(hc == HC - 1),
                                )
                        for i in range(2):
                            nc.vector.tensor_copy(
                                vres[tcq * 4 + tp + i][:], accs[i][:]
                            )

                    groups.append(v_group)
                return groups

            def attn_head(tcq, h, qT, attnT, pv_pool, den_pool):
                for _ in attn_head_gen(tcq, h, qT, attnT, pv_pool, den_pool):
                    pass

            def attn_head_gen(tcq, h, qT, attnT, pv_pool, den_pool):
                # Causal-tight: diagonal k-tile ki=4*tcq+j only attends
                # q-cols [128j, 512) of the quarter; within that only the
                # first 128 cols need the triangular mask.
                nki = 4 * (tcq + 1)
                pv_ps = pv_pool.tile([128, QT], F32, tag=pv_pool._attag, name="pv")
                den_ps = den_pool.tile([1, QT], F32, tag=den_pool._attag, name="den")
                first = [True]
                first_den = [True]
                pending_den = []

                def pv_den(ki, src_ap, col0, ncols, stop):
                    nc.tensor.matmul(
                        pv_ps[:, ds(col0, ncols)],
                        vres[ki][:, ts(h, 128)],
                        src_ap,
                        start=first[0],
                        stop=stop,
                    )
                    first[0] = False
                    nc.tensor.matmul(
                        den_ps[:, ds(col0, ncols)],
                        ones_c[:],
                        src_ap,
                        start=first_den[0],
                        stop=stop,
                    )
                    first_den[0] = False

                pending_tsum = []

                def emit_den_for(tsum):
                    def emit():
                        nc.tensor.matmul(
                            den_ps[:],
                            ones_c[:],
                            tsum[:],
                            start=first_den[0],
                            stop=False,
                        )
                        first_den[0] = False

                    return emit

                kps = list(range(nki // 2))
                for kp in kps:
                    yield
                    if kp < 2 * tcq:  # full pair
                        sc2 = ps_sc.tile([128, 2 * QT], F32, tag="sc", name="sc")
                        pt2 = ptp.tile([128, 2 * QT], BF16, tag="pt", name="pt")
                        split_exp = tcq == NQ - 1
                        for half in range(2):
                            ki = kp * 2 + half
                            nc.tensor.matmul(
                                sc2[:, ts(half, QT)],
                                kT[h][ki // 4][:, ts(ki % 4, 128)],
                                qT[h][:],
                                start=True,
                                stop=True,
                            )
                            if split_exp:
                                nc.scalar.activation(
                                    pt2[:, ts(half, QT)],
                                    sc2[:, ts(half, QT)],
                                    EXP,
                                    scale=SCALE,
                                )
                        if not split_exp:
                            nc.scalar.activation(pt2[:], sc2[:], EXP, scale=SCALE)
                        for half in range(2):
                            ki = kp * 2 + half
                            nc.tensor.matmul(
                                pv_ps[:],
                                vres[ki][:, ts(h, 128)],
                                pt2[:, ts(half, QT)],
                                start=first[0],
                                stop=False,
                            )
                            first[0] = False
                        tsum = pmp.tile([128, QT], BF16, tag="tsum", name="tsum", bufs=4)
                        nc.vector.tensor_add(
                            tsum[:], pt2[:, ts(0, QT)], pt2[:, ts(1, QT)]
                        )
                        # merge two pair-sums (4 k-tiles) per den matmul;
                        # defer so the DVE adds never stall the PE
                        pending_tsum.append(tsum)
                        if len(pending_tsum) >= 3:
                            ta = pending_tsum.pop(0)
                            tb = pending_tsum.pop(0)
                            t4 = pmp.tile(
                                [128, QT], BF16, tag="tsum4", name="tsum4", bufs=2
                            )
                            nc.vector.tensor_add(t4[:], ta[:], tb[:])
                            pending_den.append(emit_den_for(t4))
                        if len(pending_den) > 1:
                            pending_den.pop(0)()
                    else:  # diagonal pair: j pairs (0,1) then (2,3)
                        if pending_tsum:
                            if len(pending_tsum) == 2:
                                ta = pending_tsum.pop(0)
                                tb = pending_tsum.pop(0)
                                t4 = pmp.tile(
                                    [128, QT], BF16, tag="tsum4", name="tsum4", bufs=2
                                )
                                nc.vector.tensor_add(t4[:], ta[:], tb[:])
                                pending_den.append(emit_den_for(t4))
                            else:
                                pending_den.append(emit_den_for(pending_tsum.pop(0)))
                        while pending_den:
                            pending_den.pop(0)()
                        ja = 2 * (kp - 2 * tcq)  # 0 or 2
                        wa, wb = QT - 128 * ja, QT - 128 * (ja + 1)
                        sc2 = ps_sc.tile([128, wa + wb], F32, tag="sc", name="sc")
                        nc.tensor.matmul(
                            sc2[:, 0:wa],
                            kT[h][tcq][:, ts(ja, 128)],
                            qT[h][:, ds(128 * ja, wa)],
                            start=True,
                            stop=True,
                        )
                        nc.tensor.matmul(
                            sc2[:, ds(wa, wb)],
                            kT[h][tcq][:, ts(ja + 1, 128)],
                            qT[h][:, ds(128 * (ja + 1), wb)],
                            start=True,
                            stop=True,
                        )
                        pt2 = ptp.tile([128, wa + wb], BF16, tag="pt", name="pt")
                        nc.scalar.activation(pt2[:], sc2[:], EXP, scale=SCALE)
                        # mask the first 128 cols of each diagonal piece
                        # in place, then one pv + one den matmul per piece
                        for j, off, w in ((ja, 0, wa), (ja + 1, wa, wb)):
                            nc.vector.tensor_mul(
                                pt2[:, ds(off, 128)],
                                pt2[:, ds(off, 128)],
                                mask_t[:, 0:128],
                            )
                        for j, off, w in ((ja, 0, wa), (ja + 1, wa, wb)):
                            ki = 4 * tcq + j
                            pv_den(ki, pt2[:, ds(off, w)], 128 * j, w, stop=(j == 3))
                recip = rp.tile([1, QT], F32, tag="rc", name="rc")
                nc.vector.reciprocal_approx_fast(recip[:], den_ps[:])
                recip_r = rp.tile([1, QT], F32R, tag="rcr", name="rcr")
                nc.vector.tensor_copy(recip_r[:], recip[:])
                bc = den_pool.tile([128, QT], F32, tag=den_pool._attag, name="bc")
                nc.tensor.matmul(
                    bc[:], ones_r[:], recip_r[:], start=True, stop=True
                )
                bc_sb = rp.tile([128, QT], F32, tag="bcsb", name="bcsb")
                nc.vector.tensor_copy(bc_sb[:], bc[:])
                at_t = atp.tile([128, QT], BF16, tag=f"at{h}", name=f"at{h}")
                nc.vector.tensor_mul(at_t[:], pv_ps[:], bc_sb[:])
                while len(attnT) <= h:
                    attnT.append(None)
                attnT[h] = at_t

            ps_pv._attag = "pv"
            ps_den._attag = "den"
            ps_acc._attag = "acc"

            # ---- software pipeline over quarters: attention(t) interleaves
            # with the projection sweeps of quarter t+1 on the PE stream ----
            quarters = [(r, t) for r in range(reps) for t in range(NQ)]
            xs_cur = [None] * 4
            load_weight(wq_r, wqt, 0)
            xs_cur[0] = load_x(0, 0, eng=nc.sync)
            load_weight(wq_r, wqt, 1)
            xs_cur[1] = load_x(0, 1, eng=nc.gpsimd)
            xs_cur[2] = load_x(0, 2, eng=nc.sync)
            load_weight(wq_r, wqt, 2)
            xs_cur[3] = load_x(0, 3, eng=nc.gpsimd)
            load_weight(wq_r, wqt, 3)
            load_consts_weights()
            xs_pre1 = (
                [load_x(1, c) for c in range(4)] if len(quarters) > 1 else None
            )
            qT_cur = {}
            for g in sweep_groups(0, xs_cur, qT_cur):
                g()
            for qi, (rep, tcq) in enumerate(quarters):
                tsl = ts(tcq, QT)
                last = qi == len(quarters) - 1
                if not last:
                    ntc = quarters[qi + 1][1]
                    xs_next = (
                        xs_pre1 if qi == 0 else [load_x(ntc, c) for c in range(4)]
                    )
                    qT_next = {}
                    groups = sweep_groups(ntc, xs_next, qT_next)
                else:
                    groups = []

                attnT = []
                if not last:
                    for h in range(HPG):
                        attn_head(tcq, h, qT_cur, attnT, ps_pv, ps_den)
                        groups[h]()
                else:
                    # final quarter: no sweeps to overlap; run head pairs on
                    # alternating PSUM banks (borrowing the freed sweep banks)
                    # with their k-loops interleaved so neither head's PE work
                    # stalls on its own exp handoff
                    for hp in (0, 2):
                        gens = [
                            attn_head_gen(tcq, hp, qT_cur, attnT, ps_pv, ps_den),
                            attn_head_gen(
                                tcq, hp + 1, qT_cur, attnT, ps_acc, ps_acc
                            ),
                        ]
                        while gens:
                            for g in list(gens):
                                try:
                                    next(g)
                                except StopIteration:
                                    gens.remove(g)

                # ---- output projection; the first v-sweep group runs right
                # after the last attn head so its matmuls fill the PE stall
                # while that head's softmax tail (den->recip->bc) drains ----
                if not last:
                    groups[HPG]()
                for oc in range(NQ):
                    split_out = last and oc == NQ - 1
                    ost = (
                        None
                        if split_out
                        else osb.tile([128, 4, QT], BF16, tag="ost", name="ost")
                    )
                    for tt in range(4):
                        ops = ps_sc.tile([128, QT], F32, tag="sc", name="sc")
                        for h in range(HPG):
                            nc.tensor.matmul(
                                ops[:],
                                attnT[h][:, ts(tt, 128)],
                                wo_r[:, oc, h, :],
                                start=(h == 0),
                                stop=(h == HPG - 1),
                            )
                        if split_out:
                            # tail: ship each token-block as soon as its
                            # chain finishes so the final DMA is small
                            osts = osb.tile(
                                [128, QT], BF16, tag="osts", name="osts", bufs=4
                            )
                            nc.scalar.copy(osts[:], ops[:])
                            nc.sync.dma_start(
                                outq[oc, ds(tcq * QT + tt * 128, 128), :],
                                osts[:],
                            )
                        else:
                            nc.scalar.copy(ost[:, tt, :], ops[:])
                    if not split_out:
                        nc.sync.dma_start(
                            outq[oc, ts(tcq, QT), :].rearrange(
                                "(tt p) o -> p tt o", p=128
                            ),
                            ost[:],
                        )
                    if not last and oc == 0:
                        groups[HPG + 1]()

                if not last:
                    xs_cur, qT_cur = xs_next, qT_next
    nc.finalize()
    return nc


def _host_tables():
    import ml_dtypes

    inv = 1.0 / (10000.0 ** (np.arange(64, dtype=np.float64) / 64.0))
    ang = inv[:, None] * np.arange(S, dtype=np.float64)[None, :]  # [64, S]
    cosL = np.cos(ang)
    sinL = np.sin(ang)
    cos_t = np.vstack([cosL, cosL]).astype(ml_dtypes.bfloat16)
    sin_t = np.vstack([sinL, -sinL]).astype(ml_dtypes.bfloat16)
    kp = np.arange(128)[:, None]
    qf = np.arange(128)[None, :]
    mask = (qf >= kp).astype(ml_dtypes.bfloat16)
    ones = np.ones((128, 1), np.float32)
    return cos_t, sin_t, mask, ones


def _make_in_maps(hidden_states, wq, wk, wv, wo):
    import ml_dtypes

    bf16 = ml_dtypes.bfloat16
    x = np.asarray(hidden_states, dtype=np.float32)
    wq = np.asarray(wq, dtype=np.float32)
    wk = np.asarray(wk, dtype=np.float32)
    wv = np.asarray(wv, dtype=np.float32)
    wo = np.asarray(wo, dtype=np.float32)
    cos_t, sin_t, mask, ones = _host_tables()
    onesb = ones.astype(bf16)
    xqa_by_b = [
        np.ascontiguousarray(
            x[b].T.reshape(H, NQ, QT).transpose(1, 0, 2)
        ).astype(bf16)
        for b in range(B)
    ]
    w_by_g = []
    for g in range(NG):
        sl = slice(GD * g, GD * (g + 1))
        w_by_g.append(
            (
                np.ascontiguousarray(wq[sl, :].T).astype(bf16),
                np.ascontiguousarray(wk[sl, :].T).astype(bf16),
                np.ascontiguousarray(wv[sl, :].T).astype(bf16),
                # wot[oc, d, o] = wo[oc*QT + o, GD*g + d]
                np.ascontiguousarray(
                    wo[:, sl].reshape(NQ, QT, GD).transpose(0, 2, 1)
                ).astype(bf16),
            )
        )
    in_maps = []
    for c in range(8):
        b, g = divmod(c, NG)
        wqt, wkt, wvt, wot = w_by_g[g]
        in_maps.append(
            {
                "xq": xqa_by_b[b], "wqt": wqt, "wkt": wkt, "wvt": wvt,
                "wot": wot, "cosd": cos_t, "sind": sin_t, "maskd": mask,
                "onesd": ones, "onesbd": onesb,
            }
        )
    return in_maps


def _gather(results, bo):
    out = np.zeros((B, S, H), dtype=np.float32)
    for c in range(8):
        b = c // NG
        oq = results[c]["outq"].astype(np.float32)  # [NQ, S, QT]
        out[b] += np.concatenate(list(oq), axis=1)
    out += np.asarray(bo, dtype=np.float32)[None, None, :]
    return out


def kernel(hidden_states, wq, bq, wk, bk, wv, bv, wo, bo):
    in_maps = _make_in_maps(hidden_states, wq, wk, wv, wo)
    nc = build()
    res = run_bass_kernel_spmd(nc, in_maps, core_ids=list(range(8)))
    return _gather(res.results, bo)

